# revision 4
# baseline (speedup 1.0000x reference)
"""DiagonalBiLSTM Trainium2 kernel — V9: V4 + HAM keep-warm dummy matmuls in the step tail.

Full inputs in, full output out. 8-way data-parallel over the 1024 flattened
(batch, height) scan rows; both scan directions fused into one moving
dimension so matmuls keep a large free dim (per-matmul overhead makes
small-N matmuls a loss). Directions are interleaved per column
(moving index j = col*2 + dir) so the ghost-shrunk prefix [0:ng] stays
contiguous. Ghost columns are dropped once they can no longer influence an
owned row: free dim shrinks 2*254 -> 2*128 over the scan.

Math per diagonal step d (per row r, channel vector form):
    u  = Wm @ x_diag[d] + k0 @ h[r] + k1 @ h[r+1] + (b_i2s + b_s2s)
    g  = w_ih @ u + (b_ih + b_hh)          # 4*256 gate channels
    c  = sig(g_f)*c + sig(g_i)*tanh(g_g)
    h  = sig(g_o)*tanh(c)
All matmul operands bf16 (fp32 PSUM accumulation); cell state bf16.
"""

import numpy as np
import ml_dtypes

BF16 = ml_dtypes.bfloat16

B, CIN, COUT, H, W, DC = 16, 256, 256, 64, 64, 3
WD = 2 * W - 1          # 127 diagonal steps
NCORES = 8
OWN = 128               # own rows per core (2 batches)
GHOST = 126             # max redundant ghost rows
NDATA = OWN + GHOST     # 254 data cols per direction at step 0
NCOL = 256              # padded col count per dir (col 254 = always zero)
NKC = 2                 # channel chunks (256 = 2*128)
NGT = 8                 # gate m-tiles (1024 = 8*128)

_COMPILED = {}


def _ng(d):
    # valid cols per direction at step d (ghost shrink)
    return OWN + max(0, GHOST - d)


# ----------------------------------------------------------------- host prep

def _i2s_mask_np():
    oc = np.arange(COUT) % DC
    ic = np.arange(CIN) % DC
    return (ic[None, :] <= oc[:, None]).astype(np.float32)


def _wT_tiles(w, nmt):
    # [out=nmt*128, in=256] -> lhsT tile array [k=128, kc=2, mt=nmt, m=128]
    return np.ascontiguousarray(
        w.T.reshape(NKC, 128, nmt, 128).transpose(1, 0, 2, 3))


def _diag_pack(x_loc):
    """x_loc [4, 256, 64, 64] (local batches, already W-flipped for the R dir)
    -> [WD, NKC, 128, 256cols] where col j = local row (b_loc*64 + h),
    value x[b, c, h, d - h] (0 outside the diagonal band)."""
    xs = np.zeros((WD, 4, CIN, H), np.float32)
    for h in range(H):
        # diag d = h + w for w in 0..63  ->  xs[h:h+64, :, :, h]
        xs[h:h + W, :, :, h] = x_loc[:, :, h, :].transpose(2, 0, 1)
    # [WD, 4b, 256c, 64h] -> [WD, 256c, 4b*64h] -> [WD, 2, 128, 256]
    xs = xs.transpose(0, 2, 1, 3).reshape(WD, CIN, 4 * H)
    return np.ascontiguousarray(xs.reshape(WD, NKC, 128, 4 * H))


def _prep_inputs(x, w_i2s, b_i2s, w_ih, b_ih, b_hh, k0, k1, b_s2s):
    wm = w_i2s * _i2s_mask_np()
    wm_t = _wT_tiles(wm, 2).astype(BF16)
    k0_t = _wT_tiles(k0, 2).astype(BF16)
    k1_t = _wT_tiles(k1, 2).astype(BF16)
    wih_t = _wT_tiles(w_ih, 8).astype(BF16)

    bias_u = (b_i2s + b_s2s).astype(np.float32)       # [256]
    bias_g = (b_ih + b_hh).astype(np.float32)         # [1024]
    misc_all = np.zeros((NCORES, 128, 12), np.float32)
    misc_all[:, :, 0:2] = bias_u.reshape(2, 128).T[None]
    misc_all[:, :, 2:10] = bias_g.reshape(8, 128).T[None]
    misc_all[:, :, 10] = 1.0
    misc_all[7, :, 10] = 0.0                          # core 7: zero ghost0 h

    xf = x[:, :, :, ::-1]                             # W-flip for R direction
    in_maps = []
    for c in range(NCORES):
        xloc = np.zeros((4, CIN, H, W), np.float32)
        xfloc = np.zeros((4, CIN, H, W), np.float32)
        nb = min(4, B - 2 * c)
        xloc[:nb] = x[2 * c:2 * c + nb]
        xfloc[:nb] = xf[2 * c:2 * c + nb]
        dl = _diag_pack(xloc)                         # [WD, 2, 128, 256]
        dr = _diag_pack(xfloc)
        xd = np.stack([dl, dr], axis=-1)              # [WD, 2, 128, 256, 2]
        in_maps.append({
            "xd": np.ascontiguousarray(xd.astype(BF16)),
            "wm": wm_t, "k0t": k0_t, "k1t": k1_t, "wih": wih_t,
            "misc": misc_all[c],
        })
    return in_maps


# ------------------------------------------------------- reference-free host
# numpy replica of the device program, for debugging (same per-core arrays)

def _core_sim(im, nsteps=WD):
    xd = im["xd"]                   # [WD, 2, 128, 256, 2] bf16
    wm_t, k0_t, k1_t, wih_t = im["wm"], im["k0t"], im["k1t"], im["wih"]
    misc = im["misc"]

    def unT(t, nmt):                # tile array -> [out, in]
        return np.asarray(t, np.float32).transpose(1, 0, 2, 3)\
            .reshape(CIN, nmt * 128).T

    wm, k0, k1, wih = unT(wm_t, 2), unT(k0_t, 2), unT(k1_t, 2), unT(wih_t, 8)
    bias_u = misc[:, 0:2].T.reshape(CIN)
    bias_g = misc[:, 2:10].T.reshape(8 * 128)
    s = misc[0, 10]

    def sig(v):
        return 1.0 / (1.0 + np.exp(-v))

    def bf(v):
        return v.astype(BF16).astype(np.float32)

    h = np.zeros((CIN, NCOL, 2), np.float32)     # [ch, col, dir]
    cst = np.zeros((CIN, NCOL, 2), np.float32)
    out = np.zeros((WD, 2, 128, OWN, 2), BF16)
    for d in range(nsteps):
        ng = _ng(d)
        xs = np.asarray(xd[d], np.float32).reshape(CIN, NCOL, 2)[:, :ng]\
            .reshape(CIN, -1)
        hp = np.ascontiguousarray(h[:, 0:ng]).reshape(CIN, -1)
        hn = np.ascontiguousarray(h[:, 1:ng + 1]).reshape(CIN, -1)
        u = bf(wm @ xs + k0 @ hp + k1 @ hn + bias_u[:, None])
        g = wih @ u + bias_g[:, None]
        gi, gf, gg, go = g[0:256], g[256:512], g[512:768], g[768:1024]
        sgi, sgf, sgo = bf(sig(gi)), bf(sig(gf)), bf(sig(go))
        tgg = bf(np.tanh(gg))
        t1 = bf(sgi * tgg)
        cv = np.ascontiguousarray(cst[:, 0:ng]).reshape(CIN, -1)
        cv = bf(bf(cv * sgf) + t1)
        cst[:, 0:ng] = cv.reshape(CIN, ng, 2)
        t2 = bf(np.tanh(cv))
        h[:, 0:ng] = bf(sgo * t2).reshape(CIN, ng, 2)
        h[:, OWN] *= s
        out[d] = h.reshape(2, 128, NCOL, 2)[:, :, 0:OWN].astype(BF16)
    return out


# ----------------------------------------------------------- output assembly

def _assemble(core_outs):
    # core_outs: list of [WD, 2kc, 128, OWN, 2dir] -> [B, COUT, H, W]
    hs = np.zeros((2, WD, CIN, B * H), np.float32)
    for c, o in enumerate(core_outs):
        o = np.asarray(o, np.float32)
        hs[:, :, :, c * OWN:(c + 1) * OWN] = (
            o.transpose(4, 0, 1, 2, 3).reshape(2, WD, CIN, OWN))

    def unscramble(hd):             # [WD, 256ch, 1024rows] -> [B, COUT, H, WD]
        a = hd.transpose(0, 2, 1).reshape(WD, B, COUT, H)
        return a.transpose(1, 2, 3, 0)

    def unshift(a):                 # [B, COUT, H, WD] -> [B, COUT, H, W]
        rows = np.arange(H)[:, None]
        cols = rows + np.arange(W)[None, :]
        return a[:, :, rows, cols]

    left = unshift(unscramble(hs[0]))
    right = unshift(unscramble(hs[1]))[:, :, :, ::-1]
    right = np.concatenate(
        [np.zeros_like(right[:, :, :1, :]), right[:, :, :-1, :]], axis=2)
    return left + right


# --------------------------------------------------------------- bass kernel

def _build(nsteps=WD):
    import concourse.bacc as bacc
    import concourse.mybir as mybir
    import concourse.tile as tile
    from concourse._compat import get_trn_type

    f32 = mybir.dt.float32
    bf = mybir.dt.bfloat16
    AF = mybir.ActivationFunctionType

    nc = bacc.Bacc(get_trn_type() or "TRN2", target_bir_lowering=False,
                   debug=False)
    xd = nc.dram_tensor("xd", [WD, NKC, 128, NCOL, 2], bf,
                        kind="ExternalInput")
    wm = nc.dram_tensor("wm", [128, NKC, 2, 128], bf, kind="ExternalInput")
    k0t = nc.dram_tensor("k0t", [128, NKC, 2, 128], bf, kind="ExternalInput")
    k1t = nc.dram_tensor("k1t", [128, NKC, 2, 128], bf, kind="ExternalInput")
    wih = nc.dram_tensor("wih", [128, NKC, NGT, 128], bf,
                         kind="ExternalInput")
    misc = nc.dram_tensor("misc", [128, 12], f32, kind="ExternalInput")
    hs_out = nc.dram_tensor("hs", [WD, NKC, 128, OWN, 2], bf,
                            kind="ExternalOutput")

    with tile.TileContext(nc) as tc:
        with (
            tc.tile_pool(name="wpool", bufs=1) as wpool,
            tc.tile_pool(name="state", bufs=1) as state,
            tc.tile_pool(name="xpool", bufs=3) as xpool,
            tc.tile_pool(name="upool", bufs=2) as upool,
            tc.tile_pool(name="apool", bufs=2) as apool,
            tc.tile_pool(name="tpool", bufs=2) as tpool,
            tc.tile_pool(name="upsum", bufs=3, space="PSUM") as upsum,
            tc.tile_pool(name="gpsum", bufs=5, space="PSUM") as gpsum,
        ):
            wm_t = wpool.tile([128, NKC, 2, 128], bf, tag="wm")
            k0_t = wpool.tile([128, NKC, 2, 128], bf, tag="k0")
            k1_t = wpool.tile([128, NKC, 2, 128], bf, tag="k1")
            wih_t = wpool.tile([128, NKC, NGT, 128], bf, tag="wih")
            misc_t = wpool.tile([128, 12], f32, tag="misc")
            nc.sync.dma_start(wm_t[:], wm[:])
            nc.sync.dma_start(k0_t[:], k0t[:])
            nc.sync.dma_start(k1_t[:], k1t[:])
            nc.sync.dma_start(wih_t[:], wih[:])
            nc.sync.dma_start(misc_t[:], misc[:])

            zt = state.tile([1, 2 * NCOL], bf, tag="zt")
            nc.any.memset(zt[:], 0.0)
            h = state.tile([128, NKC, NCOL, 2], bf, tag="h")
            cs = state.tile([128, NKC, NCOL, 2], bf, tag="c")
            nc.any.memset(h[:], 0.0)
            nc.any.memset(cs[:], 0.0)

            for d in range(nsteps):
                ng = _ng(d)
                xs = xpool.tile([128, NKC, NCOL, 2], bf, tag="xs")
                for kc in range(NKC):
                    nc.sync.dma_start(xs[:, kc, 0:ng], xd[d, kc, :, 0:ng])

                u = upool.tile([128, NKC, NCOL, 2], bf, tag="u")
                ups = []
                for m in range(NKC):
                    up = upsum.tile([128, NCOL, 2], f32, tag="up")
                    for _ in range(2):
                        nc.tensor.matmul(
                            up[:, 0:256, :], zt[0:1, 0:128],
                            zt[0:1, 0:512],
                            start=True, stop=True, skip_group_check=True)
                    for kc in range(NKC):
                        nc.tensor.matmul(
                            up[:, 0:ng], wm_t[:, kc, m, :],
                            xs[:, kc, 0:ng],
                            start=(kc == 0), stop=False)
                    ups.append(up)
                for kc in range(NKC):
                    for m in range(NKC):
                        up = ups[m]
                        nc.tensor.matmul(
                            up[:, 0:ng], k0_t[:, kc, m, :],
                            h[:, kc, 0:ng], start=False, stop=False)
                        nc.tensor.matmul(
                            up[:, 0:ng], k1_t[:, kc, m, :],
                            h[:, kc, 1:ng + 1],
                            start=False, stop=(kc == NKC - 1))
                for m in range(NKC):
                    nc.vector.tensor_scalar_add(
                        u[:, m, 0:ng], ups[m][:, 0:ng], misc_t[:, m:m + 1])

                acts = []
                torder = [0, 4, 2, 6, 1, 5, 3, 7]   # i0,g0,f0,o0,i1,g1,f1,o1
                for j, t in enumerate(torder):
                    gp = gpsum.tile([128, NCOL, 2], f32, tag="gp")
                    for kc in range(NKC):
                        nc.tensor.matmul(
                            gp[:, 0:ng], wih_t[:, kc, t, :],
                            u[:, kc, 0:ng],
                            start=(kc == 0), stop=(kc == NKC - 1))
                    a = apool.tile([128, NCOL, 2], bf, tag=f"act{j}")
                    fn = AF.Tanh if t in (4, 5) else AF.Sigmoid
                    nc.scalar.activation(a[:, 0:ng], gp[:, 0:ng], fn,
                                         bias=misc_t[:, 2 + t:3 + t])
                    acts.append(a)

                for m in range(NKC):
                    iA, gA, fA, oA = acts[4 * m:4 * m + 4]
                    t1 = tpool.tile([128, NCOL, 2], bf, tag=f"t1_{m}")
                    nc.vector.tensor_mul(t1[:, 0:ng], iA[:, 0:ng],
                                         gA[:, 0:ng])
                    nc.vector.tensor_mul(cs[:, m, 0:ng], cs[:, m, 0:ng],
                                         fA[:, 0:ng])
                    nc.vector.tensor_add(cs[:, m, 0:ng], cs[:, m, 0:ng],
                                         t1[:, 0:ng])
                    t2 = tpool.tile([128, NCOL, 2], bf, tag=f"t2_{m}")
                    nc.scalar.activation(t2[:, 0:ng], cs[:, m, 0:ng],
                                         AF.Tanh)
                    nc.vector.tensor_mul(h[:, m, 0:ng], oA[:, 0:ng],
                                         t2[:, 0:ng])
                    nc.vector.tensor_scalar_mul(
                        h[:, m, OWN:OWN + 1], h[:, m, OWN:OWN + 1],
                        misc_t[:, 10:11])

                for kc in range(NKC):
                    nc.sync.dma_start(hs_out[d, kc], h[:, kc, 0:OWN])

    nc.finalize()
    return nc


def _get_compiled(nsteps=WD):
    if nsteps not in _COMPILED:
        _COMPILED[nsteps] = _build(nsteps)
    return _COMPILED[nsteps]


# ------------------------------------------------------------------- driver

def kernel(x, w_i2s, b_i2s, w_ih, b_ih, b_hh, k0, k1, b_s2s):
    from concourse.bass_utils import run_bass_kernel_spmd

    in_maps = _prep_inputs(np.asarray(x, np.float32), np.asarray(w_i2s),
                           np.asarray(b_i2s), np.asarray(w_ih),
                           np.asarray(b_ih), np.asarray(b_hh),
                           np.asarray(k0), np.asarray(k1), np.asarray(b_s2s))
    nc = _get_compiled()
    res = run_bass_kernel_spmd(nc, in_maps, list(range(NCORES)))
    return _assemble([res.results[c]["hs"] for c in range(NCORES)])


def kernel_numpy(x, w_i2s, b_i2s, w_ih, b_ih, b_hh, k0, k1, b_s2s):
    """Host-only replica of the device program (debug path)."""
    in_maps = _prep_inputs(np.asarray(x, np.float32), np.asarray(w_i2s),
                           np.asarray(b_i2s), np.asarray(w_ih),
                           np.asarray(b_ih), np.asarray(b_hh),
                           np.asarray(k0), np.asarray(k1), np.asarray(b_s2s))
    return _assemble([_core_sim(im) for im in in_maps])
